# revision 3
# baseline (speedup 1.0000x reference)
"""Trainium2 Bass kernel for CharPredictorMultirateFFN.

Model: emb = emb_table[tokens]; conv = relu(causal_conv1d(emb, K=16) + b);
logits = cat(emb, conv) @ lin_w.T + lin_b; out = softmax(logits).

Key algebraic restructure (tokens take only V=256 values):
  conv[s, h] = sum_k U[tok[s-15+k], k, h]   with U[v,k,h] = sum_e emb[v,e] conv_w[h,e,k]
so the conv becomes 16 shifted one-hot matmuls with contract dim 256 (half the
FLOPs of the direct E=512 conv) and the one-hot operand is exact in bf16.
The emb half of the final linear folds into P1 = emb_table @ lin_w[:, :E].T
(one-hot matmul, [256,256]), removing the embedding gather entirely.

Sharding: data-parallel over batch — 4 sequences per core on 8 cores, all
tables replicated, no collectives. Each core: ~70 GF bf16 on the PE.

biases are folded host-side: conv_b into U[:, K-1, :] (tap k=15 is always
valid for every output position), lin_b into P1 rows (shift-0 one-hot always
valid), so the device kernel has no bias adds.
"""

import numpy as np
import ml_dtypes

B, S, V, E, H, K = 32, 2048, 256, 512, 1024, 16
NCORES = 8
SEQ_PER_CORE = B // NCORES            # 4
PAD = K - 1                           # 15
SPAD = S + PAD                        # 2063
H8 = H // 128                         # 8
NTT = S // 512                        # 4 token-tiles of 512 per sequence
BF16 = ml_dtypes.bfloat16

TRACE = False          # set True (e.g. from test.py) to capture NTFF profile
SPLIT_TABLES = False   # hi/lo bf16 split of tables for ~fp32 accuracy (2x MMs)
LAST_RESULT = None     # BassKernelResults of the most recent run

_NC_CACHE = {}


def _build_nc(seq_per_core=SEQ_PER_CORE, ntt=NTT, split=False):
    """Build the Bass module (SPMD, identical program on every core)."""
    from contextlib import ExitStack
    import concourse.bacc as bacc
    import concourse.tile as tile
    import concourse.mybir as mybir

    f32 = mybir.dt.float32
    bf16 = mybir.dt.bfloat16
    AF = mybir.ActivationFunctionType
    toks = seq_per_core * ntt * 512
    nsplit = 2 if split else 1

    nc = bacc.Bacc("TRN2", target_bir_lowering=False, debug=False,
                   num_devices=NCORES)

    oh_d = nc.dram_tensor("oh", [128, 2, seq_per_core, SPAD], bf16,
                          kind="ExternalInput").ap()
    u_d = nc.dram_tensor("u", [128, nsplit, 2, K, H], bf16,
                         kind="ExternalInput").ap()
    w2_d = nc.dram_tensor("w2", [128, nsplit, H8, V], bf16,
                          kind="ExternalInput").ap()
    p1_d = nc.dram_tensor("p1", [128, nsplit, 2, V], bf16,
                          kind="ExternalInput").ap()
    out_d = nc.dram_tensor("out", [toks, V], f32, kind="ExternalOutput").ap()

    with tile.TileContext(nc) as tc, ExitStack() as ctx:
        consts = ctx.enter_context(tc.tile_pool(name="consts", bufs=1))
        u_t = consts.tile([128, nsplit, 2, K, H], bf16, name="u_t")
        nc.sync.dma_start(u_t[:], u_d[:])
        oh_t = consts.tile([128, 2, seq_per_core, SPAD], bf16, name="oh_t")
        nc.sync.dma_start(oh_t[:], oh_d[:])
        w2_t = consts.tile([128, nsplit, H8, V], bf16, name="w2_t")
        nc.sync.dma_start(w2_t[:], w2_d[:])
        p1_t = consts.tile([128, nsplit, 2, V], bf16, name="p1_t")
        nc.sync.dma_start(p1_t[:], p1_d[:])

        r_pool = ctx.enter_context(tc.tile_pool(name="rp", bufs=3))
        cps = ctx.enter_context(tc.tile_pool(name="cps", bufs=6, space="PSUM"))
        lps = ctx.enter_context(tc.tile_pool(name="lps", bufs=2, space="PSUM"))
        sm_pool = ctx.enter_context(tc.tile_pool(name="smp", bufs=4))
        out_pool = ctx.enter_context(tc.tile_pool(name="outp", bufs=4))

        def conv_emit(b, tt):
            """Conv for 512 tokens -> relu -> bf16 R tile [128, H8, 512]."""
            rt = r_pool.tile([128, H8, 512], bf16, name="rt", tag="rt")
            col0 = tt * 512
            for g in range(2):           # 4 PSUM banks per group of 4 h-chunks
                ps = [cps.tile([128, 512], f32, name=f"cp{i}", tag="cp")
                      for i in range(4)]
                nmm = K * 2 * nsplit
                i_mm = 0
                for k in range(K):
                    for vh in range(2):
                        rhs = oh_t[:, vh, b, col0 + k: col0 + k + 512]
                        for sp in range(nsplit):
                            for i in range(4):
                                h8 = g * 4 + i
                                nc.tensor.matmul(
                                    ps[i][:],
                                    u_t[:, sp, vh, k, h8 * 128:(h8 + 1) * 128],
                                    rhs,
                                    start=(i_mm == 0), stop=(i_mm == nmm - 1))
                            i_mm += 1
                for i in range(4):
                    nc.scalar.activation(rt[:, g * 4 + i, :], ps[i][:], AF.Relu)
            return rt

        def stage3_emit(b, tt, rt):
            """logits = OH@P1 + R@W2T per 128 tokens, softmax, DMA out."""
            for m in range(4):
                psl = lps.tile([128, V], f32, name="psl", tag="psl")
                col0 = PAD + tt * 512 + m * 128
                first = True
                for sp in range(nsplit):
                    for vh in range(2):
                        nc.tensor.matmul(
                            psl[:], oh_t[:, vh, b, col0:col0 + 128],
                            p1_t[:, sp, vh, :], start=first, stop=False)
                        first = False
                for sp in range(nsplit):
                    for h8 in range(H8):
                        last = (sp == nsplit - 1) and (h8 == H8 - 1)
                        nc.tensor.matmul(
                            psl[:], rt[:, h8, m * 128:(m + 1) * 128],
                            w2_t[:, sp, h8, :], start=False, stop=last)
                et = sm_pool.tile([128, V], f32, name="et", tag="et")
                ssum = sm_pool.tile([128, 1], f32, name="ssum", tag="ssum")
                nc.scalar.activation(et[:], psl[:], AF.Exp, accum_out=ssum[:])
                rec = sm_pool.tile([128, 1], f32, name="rec", tag="rec")
                nc.vector.reciprocal(rec[:], ssum[:])
                ot = out_pool.tile([128, V], f32, name="ot", tag="ot")
                nc.vector.tensor_scalar_mul(ot[:], et[:], rec[:])
                row0 = (b * ntt + tt) * 512 + m * 128
                nc.sync.dma_start(out_d[row0:row0 + 128, :], ot[:])

        # software pipeline: stage3 of tile i runs on the PE while ACT is
        # still free to relu tile i+1's PSUM -> no PE stall on the relu.
        tiles = [(b, tt) for b in range(seq_per_core) for tt in range(ntt)]
        prev = None
        for (b, tt) in tiles:
            rt = conv_emit(b, tt)
            if prev is not None:
                stage3_emit(*prev)
            prev = (b, tt, rt)
        stage3_emit(*prev)

    nc.compile()
    return nc


def _get_nc(split):
    key = ("full", split)
    if key not in _NC_CACHE:
        _NC_CACHE[key] = _build_nc(split=split)
    return _NC_CACHE[key]


def _hilo(x):
    hi = x.astype(BF16)
    lo = (x - hi.astype(np.float32)).astype(BF16)
    return np.stack([hi, lo], axis=1)  # split axis right after partition dim


def _pack_tables(emb_table, conv_w, conv_b, lin_w, lin_b, split):
    """Host-side table precompute + bf16 packing (a weight repack; ~4 GFLOP)."""
    emb_table = np.asarray(emb_table, np.float32)
    conv_w = np.asarray(conv_w, np.float32)
    lin_w = np.asarray(lin_w, np.float32)
    # U[v,k,h] = sum_e emb[v,e] * conv_w[h,e,k]
    U = (emb_table @ conv_w.transpose(1, 0, 2).reshape(E, H * K))
    U = U.reshape(V, H, K).transpose(0, 2, 1).copy()       # [V, K, H]
    U[:, K - 1, :] += np.asarray(conv_b, np.float32)
    P1 = emb_table @ lin_w[:, :E].T + np.asarray(lin_b, np.float32)[None, :]
    W2T = lin_w[:, E:].T.copy()                            # [H, V]

    u_p = U.reshape(2, 128, K, H).transpose(1, 0, 2, 3)    # [128, 2, K, H]
    p1_p = P1.reshape(2, 128, V).transpose(1, 0, 2)        # [128, 2, V]
    w2_p = W2T.reshape(H8, 128, V).transpose(1, 0, 2)      # [128, H8, V]
    if split:
        u_h = _hilo(u_p)                                   # [128, 2, 2, K, H]
        p1_h = _hilo(p1_p)
        w2_h = _hilo(w2_p)
    else:
        u_h = u_p.astype(BF16)[:, None]
        p1_h = p1_p.astype(BF16)[:, None]
        w2_h = w2_p.astype(BF16)[:, None]
    return (np.ascontiguousarray(u_h), np.ascontiguousarray(w2_h),
            np.ascontiguousarray(p1_h))


def _onehot(tokens):
    """[128, 2, B, SPAD] bf16, left-padded with 15 zero columns per sequence."""
    tok = np.asarray(tokens).astype(np.int64)
    oh = np.zeros((128, 2, B, SPAD), BF16)
    t = tok.ravel()
    b_idx = np.repeat(np.arange(B), S)
    col = np.tile(np.arange(S), B) + PAD
    oh[t % 128, t // 128, b_idx, col] = 1
    return oh


def kernel(input_sequence, emb_table, conv_w, conv_b, lin_w, lin_b):
    global LAST_RESULT
    from concourse.bass_utils import run_bass_kernel_spmd

    split = SPLIT_TABLES
    u_h, w2_h, p1_h = _pack_tables(emb_table, conv_w, conv_b, lin_w, lin_b,
                                   split)
    oh_full = _onehot(input_sequence)

    in_maps = []
    for c in range(NCORES):
        in_maps.append({
            "oh": np.ascontiguousarray(
                oh_full[:, :, c * SEQ_PER_CORE:(c + 1) * SEQ_PER_CORE, :]),
            "u": u_h, "w2": w2_h, "p1": p1_h,
        })

    nc = _get_nc(split)
    res = run_bass_kernel_spmd(nc, in_maps, core_ids=list(range(NCORES)),
                               trace=TRACE)
    LAST_RESULT = res
    outs = [res.results[c]["out"] for c in range(NCORES)]   # [8192, 256] each
    full = np.concatenate(outs, axis=0).reshape(B, S, V)
    return np.ascontiguousarray(full.astype(np.float32))


# revision 6
# speedup vs baseline: 1.0272x; 1.0272x over previous
"""Trainium2 Bass kernel for CharPredictorMultirateFFN.

Model: emb = emb_table[tokens]; conv = relu(causal_conv1d(emb, K=16) + b);
logits = cat(emb, conv) @ lin_w.T + lin_b; out = softmax(logits).

Key algebraic restructure (tokens take only V=256 values):
  conv[s, h] = sum_k U[tok[s-15+k], k, h]   with U[v,k,h] = sum_e emb[v,e] conv_w[h,e,k]
so the conv becomes 16 shifted one-hot matmuls with contract dim 256 (half the
FLOPs of the direct E=512 conv) and the one-hot operand is exact in bf16.
The emb half of the final linear folds into P1 = emb_table @ lin_w[:, :E].T
(one-hot matmul, [256,256]), removing the embedding gather entirely.

Sharding: data-parallel over batch — 4 sequences per core on 8 cores, all
tables replicated, no collectives. Each core: ~70 GF bf16 on the PE.

biases are folded host-side: conv_b into U[:, K-1, :] (tap k=15 is always
valid for every output position), lin_b into P1 rows (shift-0 one-hot always
valid), so the device kernel has no bias adds.
"""

import numpy as np
import ml_dtypes

B, S, V, E, H, K = 32, 2048, 256, 512, 1024, 16
NCORES = 8
SEQ_PER_CORE = B // NCORES            # 4
PAD = K - 1                           # 15
SPAD = S + PAD                        # 2063
H8 = H // 128                         # 8
NTT = S // 512                        # 4 token-tiles of 512 per sequence
BF16 = ml_dtypes.bfloat16

TRACE = False          # set True (e.g. from test.py) to capture NTFF profile
SPLIT_TABLES = False   # hi/lo bf16 split of tables for ~fp32 accuracy (2x MMs)
LAST_RESULT = None     # BassKernelResults of the most recent run

_NC_CACHE = {}


def _build_nc(seq_per_core=SEQ_PER_CORE, ntt=NTT, split=False):
    """Build the Bass module (SPMD, identical program on every core)."""
    from contextlib import ExitStack
    import concourse.bacc as bacc
    import concourse.tile as tile
    import concourse.mybir as mybir

    f32 = mybir.dt.float32
    bf16 = mybir.dt.bfloat16
    AF = mybir.ActivationFunctionType
    toks = seq_per_core * ntt * 512
    nsplit = 2 if split else 1

    nc = bacc.Bacc("TRN2", target_bir_lowering=False, debug=False,
                   num_devices=NCORES)

    # u layout: [part, split, h-half, vh, k, 512] so each conv group's
    # weights are a contiguous slab that can land early via sliced DMAs.
    oh_d = nc.dram_tensor("oh", [128, 2, seq_per_core, SPAD], bf16,
                          kind="ExternalInput").ap()
    u_d = nc.dram_tensor("u", [128, nsplit, 2, 2, K, H // 2], bf16,
                         kind="ExternalInput").ap()
    w2_d = nc.dram_tensor("w2", [128, nsplit, H8, V], bf16,
                          kind="ExternalInput").ap()
    p1_d = nc.dram_tensor("p1", [128, nsplit, 2, V], bf16,
                          kind="ExternalInput").ap()
    out_d = nc.dram_tensor("out", [toks, V], f32, kind="ExternalOutput").ap()

    with tile.TileContext(nc) as tc, ExitStack() as ctx:
        consts = ctx.enter_context(tc.tile_pool(name="consts", bufs=1))
        u_t = consts.tile([128, nsplit, 2, 2, K, H // 2], bf16, name="u_t")
        oh_t = consts.tile([128, 2, seq_per_core, SPAD], bf16, name="oh_t")
        w2_t = consts.tile([128, nsplit, H8, V], bf16, name="w2_t")
        p1_t = consts.tile([128, nsplit, 2, V], bf16, name="p1_t")
        # staggered loads ordered along the kernel's critical path: the
        # first conv group needs u[:, :, 0] and oh[b=0, cols<528] only.
        KQ = 4
        nc.sync.dma_start(u_t[:, :, 0, :, 0:KQ], u_d[:, :, 0, :, 0:KQ])
        nc.sync.dma_start(oh_t[:, :, 0, 0:528], oh_d[:, :, 0, 0:528])
        for kq in range(KQ, K, KQ):
            nc.sync.dma_start(u_t[:, :, 0, :, kq:kq + KQ],
                              u_d[:, :, 0, :, kq:kq + KQ])
        for kq in range(0, K, KQ):
            nc.sync.dma_start(u_t[:, :, 1, :, kq:kq + KQ],
                              u_d[:, :, 1, :, kq:kq + KQ])
        nc.sync.dma_start(oh_t[:, :, 0, 528:SPAD], oh_d[:, :, 0, 528:SPAD])
        for b in range(1, seq_per_core):
            nc.sync.dma_start(oh_t[:, :, b, :], oh_d[:, :, b, :])
        nc.sync.dma_start(w2_t[:], w2_d[:])
        nc.sync.dma_start(p1_t[:], p1_d[:])

        r_pool = ctx.enter_context(tc.tile_pool(name="rp", bufs=3))
        cps = ctx.enter_context(tc.tile_pool(name="cps", bufs=6, space="PSUM"))
        lps = ctx.enter_context(tc.tile_pool(name="lps", bufs=2, space="PSUM"))
        sm_pool = ctx.enter_context(tc.tile_pool(name="smp", bufs=4))
        out_pool = ctx.enter_context(tc.tile_pool(name="outp", bufs=4))

        def conv_emit(b, tt):
            """Conv for 512 tokens -> relu -> bf16 R tile [128, H8, 512]."""
            rt = r_pool.tile([128, H8, 512], bf16, name="rt", tag="rt")
            col0 = tt * 512
            for g in range(2):           # 4 PSUM banks per group of 4 h-chunks
                ps = [cps.tile([128, 512], f32, name=f"cp{i}", tag="cp")
                      for i in range(4)]
                nmm = K * 2 * nsplit
                i_mm = 0
                for k in range(K):
                    for vh in range(2):
                        rhs = oh_t[:, vh, b, col0 + k: col0 + k + 512]
                        for sp in range(nsplit):
                            for i in range(4):
                                nc.tensor.matmul(
                                    ps[i][:],
                                    u_t[:, sp, g, vh, k, i * 128:(i + 1) * 128],
                                    rhs,
                                    start=(i_mm == 0), stop=(i_mm == nmm - 1))
                            i_mm += 1
                for i in range(4):
                    nc.scalar.activation(rt[:, g * 4 + i, :], ps[i][:], AF.Relu)
            return rt

        def stage3_emit(b, tt, rt):
            """logits = OH@P1 + R@W2T per 128 tokens, softmax, DMA out."""
            for m in range(4):
                psl = lps.tile([128, V], f32, name="psl", tag="psl")
                col0 = PAD + tt * 512 + m * 128
                first = True
                for sp in range(nsplit):
                    for vh in range(2):
                        nc.tensor.matmul(
                            psl[:], oh_t[:, vh, b, col0:col0 + 128],
                            p1_t[:, sp, vh, :], start=first, stop=False)
                        first = False
                for sp in range(nsplit):
                    for h8 in range(H8):
                        last = (sp == nsplit - 1) and (h8 == H8 - 1)
                        nc.tensor.matmul(
                            psl[:], rt[:, h8, m * 128:(m + 1) * 128],
                            w2_t[:, sp, h8, :], start=False, stop=last)
                et = sm_pool.tile([128, V], f32, name="et", tag="et")
                ssum = sm_pool.tile([128, 1], f32, name="ssum", tag="ssum")
                nc.scalar.activation(et[:], psl[:], AF.Exp, accum_out=ssum[:])
                rec = sm_pool.tile([128, 1], f32, name="rec", tag="rec")
                nc.vector.reciprocal(rec[:], ssum[:])
                ot = out_pool.tile([128, V], f32, name="ot", tag="ot")
                nc.vector.tensor_scalar_mul(ot[:], et[:], rec[:])
                row0 = (b * ntt + tt) * 512 + m * 128
                nc.sync.dma_start(out_d[row0:row0 + 128, :], ot[:])

        # software pipeline: stage3 of tile i runs on the PE while ACT is
        # still free to relu tile i+1's PSUM -> no PE stall on the relu.
        tiles = [(b, tt) for b in range(seq_per_core) for tt in range(ntt)]
        prev = None
        for (b, tt) in tiles:
            rt = conv_emit(b, tt)
            if prev is not None:
                stage3_emit(*prev)
            prev = (b, tt, rt)
        stage3_emit(*prev)

    nc.compile()
    return nc


def _get_nc(split):
    key = ("full", split)
    if key not in _NC_CACHE:
        _NC_CACHE[key] = _build_nc(split=split)
    return _NC_CACHE[key]


def _hilo(x):
    hi = x.astype(BF16)
    lo = (x - hi.astype(np.float32)).astype(BF16)
    return np.stack([hi, lo], axis=1)  # split axis right after partition dim


def _pack_tables(emb_table, conv_w, conv_b, lin_w, lin_b, split):
    """Host-side table precompute + bf16 packing (a weight repack; ~4 GFLOP)."""
    emb_table = np.asarray(emb_table, np.float32)
    conv_w = np.asarray(conv_w, np.float32)
    lin_w = np.asarray(lin_w, np.float32)
    # U[v,k,h] = sum_e emb[v,e] * conv_w[h,e,k]
    U = (emb_table @ conv_w.transpose(1, 0, 2).reshape(E, H * K))
    U = U.reshape(V, H, K).transpose(0, 2, 1).copy()       # [V, K, H]
    U[:, K - 1, :] += np.asarray(conv_b, np.float32)
    P1 = emb_table @ lin_w[:, :E].T + np.asarray(lin_b, np.float32)[None, :]
    W2T = lin_w[:, E:].T.copy()                            # [H, V]

    # [128, hh, vh, K, 512]: u_p[p, hh, vh, k, c] = U[vh*128+p, k, hh*512+c]
    u_p = (U.reshape(2, 128, K, 2, H // 2)
           .transpose(1, 3, 0, 2, 4))
    p1_p = P1.reshape(2, 128, V).transpose(1, 0, 2)        # [128, 2, V]
    w2_p = W2T.reshape(H8, 128, V).transpose(1, 0, 2)      # [128, H8, V]
    if split:
        u_h = _hilo(u_p)                                   # [128, 2, 2, K, H]
        p1_h = _hilo(p1_p)
        w2_h = _hilo(w2_p)
    else:
        u_h = u_p.astype(BF16)[:, None]
        p1_h = p1_p.astype(BF16)[:, None]
        w2_h = w2_p.astype(BF16)[:, None]
    return (np.ascontiguousarray(u_h), np.ascontiguousarray(w2_h),
            np.ascontiguousarray(p1_h))


def _onehot(tokens):
    """[128, 2, B, SPAD] bf16, left-padded with 15 zero columns per sequence."""
    tok = np.asarray(tokens).astype(np.int64)
    oh = np.zeros((128, 2, B, SPAD), BF16)
    t = tok.ravel()
    b_idx = np.repeat(np.arange(B), S)
    col = np.tile(np.arange(S), B) + PAD
    oh[t % 128, t // 128, b_idx, col] = 1
    return oh


def kernel(input_sequence, emb_table, conv_w, conv_b, lin_w, lin_b):
    global LAST_RESULT
    from concourse.bass_utils import run_bass_kernel_spmd

    split = SPLIT_TABLES
    u_h, w2_h, p1_h = _pack_tables(emb_table, conv_w, conv_b, lin_w, lin_b,
                                   split)
    oh_full = _onehot(input_sequence)

    in_maps = []
    for c in range(NCORES):
        in_maps.append({
            "oh": np.ascontiguousarray(
                oh_full[:, :, c * SEQ_PER_CORE:(c + 1) * SEQ_PER_CORE, :]),
            "u": u_h, "w2": w2_h, "p1": p1_h,
        })

    nc = _get_nc(split)
    res = run_bass_kernel_spmd(nc, in_maps, core_ids=list(range(NCORES)),
                               trace=TRACE)
    LAST_RESULT = res
    outs = [res.results[c]["out"] for c in range(NCORES)]   # [8192, 256] each
    full = np.concatenate(outs, axis=0).reshape(B, S, V)
    return np.ascontiguousarray(full.astype(np.float32))


# revision 10
# speedup vs baseline: 1.0296x; 1.0024x over previous
"""Trainium2 Bass kernel for CharPredictorMultirateFFN.

Model: emb = emb_table[tokens]; conv = relu(causal_conv1d(emb, K=16) + b);
logits = cat(emb, conv) @ lin_w.T + lin_b; out = softmax(logits).

Key algebraic restructure (tokens take only V=256 values):
  conv[s, h] = sum_k U[tok[s-15+k], k, h]   with U[v,k,h] = sum_e emb[v,e] conv_w[h,e,k]
so the conv becomes 16 shifted one-hot matmuls with contract dim 256 (half the
FLOPs of the direct E=512 conv) and the one-hot operand is exact in bf16.
The emb half of the final linear folds into P1 = emb_table @ lin_w[:, :E].T
(one-hot matmul, [256,256]), removing the embedding gather entirely.

Sharding: data-parallel over batch — 4 sequences per core on 8 cores, all
tables replicated, no collectives. Each core: ~70 GF bf16 on the PE.

biases are folded host-side: conv_b into U[:, K-1, :] (tap k=15 is always
valid for every output position), lin_b into P1 rows (shift-0 one-hot always
valid), so the device kernel has no bias adds.
"""

import numpy as np
import ml_dtypes

B, S, V, E, H, K = 32, 2048, 256, 512, 1024, 16
NCORES = 8
SEQ_PER_CORE = B // NCORES            # 4
PAD = K - 1                           # 15
SPAD = S + PAD                        # 2063
H8 = H // 128                         # 8
NTT = S // 512                        # 4 token-tiles of 512 per sequence
BF16 = ml_dtypes.bfloat16

TRACE = False          # set True (e.g. from test.py) to capture NTFF profile
SPLIT_TABLES = False   # hi/lo bf16 split of tables for ~fp32 accuracy (2x MMs)
LAST_RESULT = None     # BassKernelResults of the most recent run

_NC_CACHE = {}


def _build_nc(seq_per_core=SEQ_PER_CORE, ntt=NTT, split=False):
    """Build the Bass module (SPMD, identical program on every core)."""
    from contextlib import ExitStack
    import concourse.bacc as bacc
    import concourse.tile as tile
    import concourse.mybir as mybir

    f32 = mybir.dt.float32
    bf16 = mybir.dt.bfloat16
    AF = mybir.ActivationFunctionType
    toks = seq_per_core * ntt * 512
    nsplit = 2 if split else 1

    nc = bacc.Bacc("TRN2", target_bir_lowering=False, debug=False,
                   num_devices=NCORES)

    # u layout: [part, split, h-half, k, vh, 512] so each conv group's
    # weights are contiguous per-k slabs that can land early via sliced DMAs.
    oh_d = nc.dram_tensor("oh", [128, 2, seq_per_core, SPAD], bf16,
                          kind="ExternalInput").ap()
    u_d = nc.dram_tensor("u", [128, nsplit, 2, K, 2, H // 2], bf16,
                         kind="ExternalInput").ap()
    w2_d = nc.dram_tensor("w2", [128, nsplit, H8, V], bf16,
                          kind="ExternalInput").ap()
    p1_d = nc.dram_tensor("p1", [128, nsplit, 2, V], bf16,
                          kind="ExternalInput").ap()
    out_d = nc.dram_tensor("out", [toks, V], f32, kind="ExternalOutput").ap()

    with tile.TileContext(nc) as tc, ExitStack() as ctx:
        consts = ctx.enter_context(tc.tile_pool(name="consts", bufs=1))
        u_t = consts.tile([128, nsplit, 2, K, 2, H // 2], bf16, name="u_t")
        oh_t = consts.tile([128, 2, seq_per_core, SPAD], bf16, name="oh_t")
        w2_t = consts.tile([128, nsplit, H8, V], bf16, name="w2_t")
        p1_t = consts.tile([128, nsplit, 2, V], bf16, name="p1_t")
        # staggered loads ordered along the kernel's critical path: the
        # first conv group consumes u[:, :, 0, k] in k order on oh[b=0,
        # cols<528], so stream those slabs first in small chunks.
        nc.sync.dma_start(oh_t[:, :, 0, 0:528], oh_d[:, :, 0, 0:528])
        KQ = 2
        for kq in range(0, K, KQ):
            nc.sync.dma_start(u_t[:, :, 0, kq:kq + KQ],
                              u_d[:, :, 0, kq:kq + KQ])
        for kq in range(0, K, KQ):
            nc.sync.dma_start(u_t[:, :, 1, kq:kq + KQ],
                              u_d[:, :, 1, kq:kq + KQ])
        nc.sync.dma_start(oh_t[:, :, 0, 528:SPAD], oh_d[:, :, 0, 528:SPAD])
        for b in range(1, seq_per_core):
            nc.sync.dma_start(oh_t[:, :, b, :], oh_d[:, :, b, :])
        nc.sync.dma_start(w2_t[:], w2_d[:])
        nc.sync.dma_start(p1_t[:], p1_d[:])

        r_pool = ctx.enter_context(tc.tile_pool(name="rp", bufs=3))
        cps = ctx.enter_context(tc.tile_pool(name="cps", bufs=6, space="PSUM"))
        lps = ctx.enter_context(tc.tile_pool(name="lps", bufs=2, space="PSUM"))
        sm_pool = ctx.enter_context(tc.tile_pool(name="smp", bufs=4))
        out_pool = ctx.enter_context(tc.tile_pool(name="outp", bufs=4))

        def conv_emit(b, tt):
            """Conv for 512 tokens -> relu -> bf16 R tile [128, H8, 512]."""
            rt = r_pool.tile([128, H8, 512], bf16, name="rt", tag="rt")
            col0 = tt * 512
            for g in range(2):           # 4 PSUM banks per group of 4 h-chunks
                ps = [cps.tile([128, 512], f32, name=f"cp{i}", tag="cp")
                      for i in range(4)]
                nmm = K * 2 * nsplit
                i_mm = 0
                for k in range(K):
                    for vh in range(2):
                        rhs = oh_t[:, vh, b, col0 + k: col0 + k + 512]
                        for sp in range(nsplit):
                            for i in range(4):
                                nc.tensor.matmul(
                                    ps[i][:],
                                    u_t[:, sp, g, k, vh, i * 128:(i + 1) * 128],
                                    rhs,
                                    start=(i_mm == 0), stop=(i_mm == nmm - 1))
                            i_mm += 1
                for i in range(4):
                    nc.scalar.activation(rt[:, g * 4 + i, :], ps[i][:], AF.Relu)
            return rt

        def stage3_emit(b, tt, rt):
            """logits = OH@P1 + R@W2T per 128 tokens, softmax, DMA out."""
            for m in range(4):
                psl = lps.tile([128, V], f32, name="psl", tag="psl")
                col0 = PAD + tt * 512 + m * 128
                first = True
                for sp in range(nsplit):
                    for vh in range(2):
                        nc.tensor.matmul(
                            psl[:], oh_t[:, vh, b, col0:col0 + 128],
                            p1_t[:, sp, vh, :], start=first, stop=False)
                        first = False
                for sp in range(nsplit):
                    for h8 in range(H8):
                        last = (sp == nsplit - 1) and (h8 == H8 - 1)
                        nc.tensor.matmul(
                            psl[:], rt[:, h8, m * 128:(m + 1) * 128],
                            w2_t[:, sp, h8, :], start=False, stop=last)
                et = sm_pool.tile([128, V], f32, name="et", tag="et")
                ssum = sm_pool.tile([128, 1], f32, name="ssum", tag="ssum")
                nc.scalar.activation(et[:], psl[:], AF.Exp, accum_out=ssum[:])
                rec = sm_pool.tile([128, 1], f32, name="rec", tag="rec")
                nc.vector.reciprocal(rec[:], ssum[:])
                ot = out_pool.tile([128, V], f32, name="ot", tag="ot")
                nc.vector.tensor_scalar_mul(ot[:], et[:], rec[:])
                row0 = (b * ntt + tt) * 512 + m * 128
                nc.sync.dma_start(out_d[row0:row0 + 128, :], ot[:])

        # software pipeline: stage3 of tile i runs on the PE while ACT is
        # still free to relu tile i+1's PSUM -> no PE stall on the relu.
        tiles = [(b, tt) for b in range(seq_per_core) for tt in range(ntt)]
        prev = None
        for (b, tt) in tiles:
            rt = conv_emit(b, tt)
            if prev is not None:
                stage3_emit(*prev)
            prev = (b, tt, rt)
        stage3_emit(*prev)

    nc.compile()
    return nc


def _get_nc(split):
    key = ("full", split)
    if key not in _NC_CACHE:
        _NC_CACHE[key] = _build_nc(split=split)
    return _NC_CACHE[key]


def _hilo(x):
    hi = x.astype(BF16)
    lo = (x - hi.astype(np.float32)).astype(BF16)
    return np.stack([hi, lo], axis=1)  # split axis right after partition dim


def _pack_tables(emb_table, conv_w, conv_b, lin_w, lin_b, split):
    """Host-side table precompute + bf16 packing (a weight repack; ~4 GFLOP)."""
    emb_table = np.asarray(emb_table, np.float32)
    conv_w = np.asarray(conv_w, np.float32)
    lin_w = np.asarray(lin_w, np.float32)
    # U[v,k,h] = sum_e emb[v,e] * conv_w[h,e,k]
    U = (emb_table @ conv_w.transpose(1, 0, 2).reshape(E, H * K))
    U = U.reshape(V, H, K).transpose(0, 2, 1).copy()       # [V, K, H]
    U[:, K - 1, :] += np.asarray(conv_b, np.float32)
    P1 = emb_table @ lin_w[:, :E].T + np.asarray(lin_b, np.float32)[None, :]
    W2T = lin_w[:, E:].T.copy()                            # [H, V]

    # [128, hh, K, vh, 512]: u_p[p, hh, k, vh, c] = U[vh*128+p, k, hh*512+c]
    u_p = (U.reshape(2, 128, K, 2, H // 2)
           .transpose(1, 3, 2, 0, 4))
    p1_p = P1.reshape(2, 128, V).transpose(1, 0, 2)        # [128, 2, V]
    w2_p = W2T.reshape(H8, 128, V).transpose(1, 0, 2)      # [128, H8, V]
    if split:
        u_h = _hilo(u_p)                                   # [128, 2, 2, K, H]
        p1_h = _hilo(p1_p)
        w2_h = _hilo(w2_p)
    else:
        u_h = u_p.astype(BF16)[:, None]
        p1_h = p1_p.astype(BF16)[:, None]
        w2_h = w2_p.astype(BF16)[:, None]
    return (np.ascontiguousarray(u_h), np.ascontiguousarray(w2_h),
            np.ascontiguousarray(p1_h))


def _onehot(tokens):
    """[128, 2, B, SPAD] bf16, left-padded with 15 zero columns per sequence."""
    tok = np.asarray(tokens).astype(np.int64)
    oh = np.zeros((128, 2, B, SPAD), BF16)
    t = tok.ravel()
    b_idx = np.repeat(np.arange(B), S)
    col = np.tile(np.arange(S), B) + PAD
    oh[t % 128, t // 128, b_idx, col] = 1
    return oh


def kernel(input_sequence, emb_table, conv_w, conv_b, lin_w, lin_b):
    global LAST_RESULT
    from concourse.bass_utils import run_bass_kernel_spmd

    split = SPLIT_TABLES
    u_h, w2_h, p1_h = _pack_tables(emb_table, conv_w, conv_b, lin_w, lin_b,
                                   split)
    oh_full = _onehot(input_sequence)

    in_maps = []
    for c in range(NCORES):
        in_maps.append({
            "oh": np.ascontiguousarray(
                oh_full[:, :, c * SEQ_PER_CORE:(c + 1) * SEQ_PER_CORE, :]),
            "u": u_h, "w2": w2_h, "p1": p1_h,
        })

    nc = _get_nc(split)
    res = run_bass_kernel_spmd(nc, in_maps, core_ids=list(range(NCORES)),
                               trace=TRACE)
    LAST_RESULT = res
    outs = [res.results[c]["out"] for c in range(NCORES)]   # [8192, 256] each
    full = np.concatenate(outs, axis=0).reshape(B, S, V)
    return np.ascontiguousarray(full.astype(np.float32))
